# revision 1
# baseline (speedup 1.0000x reference)
"""BERT self-attention layer (B=8, S=1024, H=12, Dh=64) on 8 trn2 NeuronCores.

Sharding: pure data-parallel over batch (1 batch item per core, weights
replicated).

Matmul path runs in fp8e4m3 with DoubleRow perf mode where the contraction
is >=256 (QKV projections 6->3 passes, ctx 8->4, dense 6->3).  Scores stay
single-pass fp8 (K=64).  The residual + LN path stays exact fp32; the
residual dominates the output (dense branch is ~1% of it), so fp8 noise in
the attention path dilutes ~100x and the final error is ~1e-4 relative.

Layouts (T = features on partitions):
  x_all  [128, 6*1024] fp8  xT, col = kt*1024 + q        (DR pairs via view)
  w*_all [128, 6*768]  fp8  W^T, col = kt*768 + fo
  qT/kT  6 x [128, 1024] fp8 per head-pair
  vpair  [128, 4*2*1536] fp8  V natural in 128-wide head blocks:
         col = jp*3072 + (j%2)*1536 + 128*h + c; c in 0:64 = values,
         c=64 = ones (denominator row), 65:127 = filler ones feeding unread
         psum rows (DoubleRow ldweights requires stationary width 32/64/128)
  e      [128, 2*1024] fp8 per (half, jp): exp(scores) pairs for DR ctx
  ctx_all[128, 6*1024] fp8  normalized ctx^T, col = kt*1024 + q

Per-core dataflow:
  loads: everything row-major (big DMA packets; the 32x32-permuted loads
         were DMA-packet-bound at ~40us/weight), spread over the three
         DMA-capable queue rings (sync / scalar / gpsimd)
  xT   = ScalarE fp8 cast + PE fp8 transpose (1 cyc/col, psum elem step 2),
         ScalarE evac
  w^T  = DVE fp8 cast + PE fp8 transpose, DVE evac (all four weights up
         front; attention needs all 8 psum banks)
  QT/KT= DR(w^T, xT) per head-pair, DVE evac
  V    = DR(xT, wv^T) natural layout, DVE strided evac into vpair
  per head pair, per j: sT = K^T-slice @ QT (fp8, psum), e = exp(sT/8+mask)
  ctx  = DR(vpair, e-pairs) accumulated over 4 jp, deferred one jp so the
         matmuls hide under the next exps -> [128, S]; row 64 = denominator
  ctxT = cc[0:64] * bcast(1/sbuf-copy(cc[64]))  (the custom-DVE reciprocal
         reads garbage from PSUM, so copy to SBUF first) -> fp8
  out  = LN(x + DR(ctxT, wd^T))  fused via STT/accum_out
"""

import os
import numpy as np
from contextlib import ExitStack

import concourse.bass as bass
import concourse.bacc as bacc
import concourse.tile as tile
from concourse import mybir
from concourse._compat import with_exitstack
from concourse.bass import ts, ds
from concourse.bass_utils import run_bass_kernel_spmd
from concourse.masks import make_identity

H = 12
DH = 64
D = 768
S = 1024
P = 128
KT_ = D // P  # 6 feature tiles
ST_ = S // P  # 8 sequence tiles
HB = 128  # per-head V block width: 64 value cols + ones col at 64 + 63
          # garbage pad cols (DoubleRow ldweights requires stationary width
          # of exactly 32/64/128; psum rows 65-127 are never read)
VW = H * HB  # 1536
EPS = 1e-12
F32 = mybir.dt.float32
FP8 = mybir.dt.float8e4  # e4m3
U8 = mybir.dt.uint8
FT = mybir.ActivationFunctionType
ALU = mybir.AluOpType
DR = mybir.MatmulPerfMode.DoubleRow
N_CORES = 8
ONE_FP8 = 0x38  # fp8e4m3 encoding of 1.0


def _copy(eng, out, in_):
    # ScalarE spells its copy differently from the vector engines
    if hasattr(eng, "tensor_copy"):
        eng.tensor_copy(out, in_)
    else:
        eng.copy(out, in_)


def _w_dma(nc, scratch, w_ap, dma_eng, tag):
    """Issue the row-major weight DMAs. Call EARLY: issues occupy the
    issuing engine's instruction stream, so they must precede its compute
    work or the transfers start tens of us late."""
    nat = [scratch.tile([P, D], F32, tag="wnat", bufs=3 * KT_,
                        name=f"{tag}n{r}") for r in range(KT_)]
    engs = dma_eng if isinstance(dma_eng, list) else [dma_eng]
    for r in range(KT_):
        engs[r % len(engs)].dma_start(out=nat[r], in_=w_ap[ts(r, P), :])
    return nat


def _load_w8T(nc, dest_all, scratch, psum_pool, nat, ident8, tag,
              cast_eng, evac_eng):
    """Load a [768, 768] f32 DRAM weight row-major (fast, few DMA packets),
    cast to fp8 (cast_eng: any vector engine, SBUF only), PE-transpose in
    fp8 (1 cyc/col; psum output needs element step 2), evac (evac_eng: must
    be DVE or ScalarE — gpsimd can't read PSUM) into dest_all
    ([128, 6*768] fp8, col=kt*768+fo)."""
    n8 = [scratch.tile([P, D], FP8, tag="wn8", bufs=2 * KT_,
                       name=f"{tag}e{r}") for r in range(KT_)]
    for r in range(KT_):
        _copy(cast_eng, n8[r], nat[r])
    for c in range(KT_):
        tp = psum_pool.tile([P, 2 * D], FP8, tag="tp8", bufs=2, name="tp8")
        t4 = tp.rearrange("p (k b two) -> p k b two", two=2, b=P)
        for r in range(KT_):
            nc.tensor.transpose(t4[:, r, :, 0], n8[r][:, ts(c, P)], ident8)
        _copy(evac_eng,
              dest_all[:, ds(c * D, D)].rearrange("p (k b) -> p k b", b=P),
              t4[:, :, :, 0])


def _permuted_src(ap, col0, n_free_blocks):
    """DRAM AP enumerating src[32J+r, col0+c] for r,c in 32x32 blocks, in
    (r, J, c) order — the 32x32-block-permuted load feeding StreamTranspose."""
    rs = ap.ap[0][0]
    return bass.AP(
        tensor=ap.tensor,
        offset=ap.offset + col0,
        ap=[[rs, 32], [32 * rs, n_free_blocks], [1, 32]],
    )


def _load_wT_dve(nc, dest_all, scratch, src_ap, dma_eng, tag):
    """Background weight transpose with NO psum: permuted DMA (packet-rate
    bound, ~40us on the wire — fine for a weight needed late) -> DVE
    StreamTranspose -> DVE fp8 cast into dest_all."""
    for kt in range(KT_):
        perm = scratch.tile([P, D], F32, tag="tsp", bufs=2, name=f"{tag}p")
        p4 = perm.rearrange("(i r) (j c) -> i r j c", r=32, c=32)
        for i in range(4):
            dma_eng.dma_start(
                out=p4[i],
                in_=_permuted_src(src_ap, 128 * kt + 32 * i, D // 32),
            )
        tf = scratch.tile([P, D], F32, tag="tst", bufs=2, name=f"{tag}t")
        nc.vector.transpose(tf, perm)
        nc.vector.tensor_copy(dest_all[:, ds(kt * D, D)], tf)


def _bcast_load(nc, out_tile, vec_ap, n_part):
    """DMA a [N] DRAM vector replicated across n_part partitions."""
    src = bass.AP(
        tensor=vec_ap.tensor,
        offset=vec_ap.offset,
        ap=[[0, n_part]] + [list(d) for d in vec_ap.ap],
    )
    nc.gpsimd.dma_start(out=out_tile, in_=src)


@with_exitstack
def bert_attn_kernel(
    ctx: ExitStack,
    tc: tile.TileContext,
    out_ap: bass.AP,
    x_ap: bass.AP,
    mask_ap: bass.AP,
    wq_ap: bass.AP,
    bq_ap: bass.AP,
    wk_ap: bass.AP,
    bk_ap: bass.AP,
    wv_ap: bass.AP,
    bv_ap: bass.AP,
    wd_ap: bass.AP,
    bd_ap: bass.AP,
    g_ap: bass.AP,
    b_ap: bass.AP,
    use_mask: bool,
    use_qkv_bias: bool,
    use_dense_bias: bool,
    use_ln_affine: bool,
):
    nc = tc.nc

    # ---- persistent pools ----
    const_pool = ctx.enter_context(tc.tile_pool(name="const", bufs=1))
    big_pool = ctx.enter_context(tc.tile_pool(name="big", bufs=1))

    eps_t = const_pool.tile([P, 1], F32)
    nc.vector.memset(eps_t, EPS)
    ident = const_pool.tile([P, P], F32)
    make_identity(nc, ident)
    ident8 = const_pool.tile([P, P], FP8)
    nc.vector.tensor_copy(ident8, ident)

    maskT = None
    if use_mask:
        maskT = const_pool.tile([P, ST_], F32)
        nc.sync.dma_start(out=maskT, in_=mask_ap.rearrange("(t p) -> p t", p=P))

    bq_t = bk_t = bv_bc = None
    if use_qkv_bias:
        bq_t = const_pool.tile([P, KT_], F32)
        nc.sync.dma_start(out=bq_t, in_=bq_ap.rearrange("(t p) -> p t", p=P))
        bk_t = const_pool.tile([P, KT_], F32)
        nc.sync.dma_start(out=bk_t, in_=bk_ap.rearrange("(t p) -> p t", p=P))
        bv_bc = const_pool.tile([P, D], F32)
        _bcast_load(nc, bv_bc, bv_ap, P)
    bd_bc = None
    if use_dense_bias:
        bd_bc = const_pool.tile([P, D], F32)
        _bcast_load(nc, bd_bc, bd_ap, P)
    g_bc = b_bc = None
    if use_ln_affine:
        g_bc = const_pool.tile([P, D], F32)
        _bcast_load(nc, g_bc, g_ap, P)
        b_bc = const_pool.tile([P, D], F32)
        _bcast_load(nc, b_bc, b_ap, P)

    # persistent data tiles
    x_all = big_pool.tile([P, KT_ * S], FP8, name="x_all")
    xv = x_all.rearrange("p (k q) -> p k q", q=S)
    wq_all = big_pool.tile([P, KT_ * D], FP8, name="wq_all")
    wk_all = big_pool.tile([P, KT_ * D], FP8, name="wk_all")
    wv_all = big_pool.tile([P, KT_ * D], FP8, name="wv_all")
    qT = [big_pool.tile([P, S], FP8, name=f"qT{i}") for i in range(KT_)]
    kT = [big_pool.tile([P, S], FP8, name=f"kT{i}") for i in range(KT_)]
    vpair = big_pool.tile([P, 4 * 2 * VW], FP8, name="vpair")
    vv = vpair.rearrange("p (j t h c) -> p j t h c", t=2, h=H, c=HB)
    xn = [big_pool.tile([P, D], F32, tag="xn", bufs=ST_, name=f"xn{i}")
          for i in range(ST_)]
    ctx_all = big_pool.tile([P, KT_ * S], FP8, name="ctx_all")
    cxv = ctx_all.rearrange("p (k q) -> p k q", q=S)
    wd_all = big_pool.tile([P, KT_ * D], FP8, name="wd_all")

    # Head-block column map: 0:64 = values, 64 = ones (denominator row),
    # 65:127 = filler ones feeding unread psum rows (deterministic, no
    # uninitialized weights entering the PE).
    for jp in range(4):
        nc.gpsimd.memset(vv[:, jp, :, :, DH:HB].bitcast(U8), ONE_FP8)

    # =========== phase 1: x + weight loads (row-major, 3 queue rings) ======
    # x is striped across all three rings (it gates everything), then
    # wq/wv/wk ride one ring each; wd comes via the background permuted
    # path on sync during attention. Casts/evacs are spread over gpsimd
    # (SBUF-only), ScalarE and DVE so no single engine serializes the
    # lead-in.
    for st in range(ST_):
        nc.sync.dma_start(out=xn[st], in_=x_ap[ts(st, P), :])

    wsc_pool = ctx.enter_context(tc.tile_pool(name="wsc", bufs=1))
    wq_nat = _w_dma(nc, wsc_pool, wq_ap, nc.scalar, "wq")
    wk_nat = _w_dma(nc, wsc_pool, wk_ap, nc.gpsimd, "wk")
    wv_nat = _w_dma(nc, wsc_pool, wv_ap, [nc.scalar, nc.gpsimd], "wv")

    # =========== phase 2: transposes + QKV projections (DoubleRow) ========
    wqv = wq_all.rearrange("p (k f) -> p k f", f=D)
    wkv = wk_all.rearrange("p (k f) -> p k f", f=D)
    wvv = wv_all.rearrange("p (k f) -> p k f", f=D)

    with tc.tile_pool(name="ps_tv", bufs=1, space="PSUM") as psum_tv, \
         tc.tile_pool(name="ps_qk", bufs=2, space="PSUM") as psum_qk:

        # xT first in the PE stream (x lands before the weights): cast to
        # fp8 on gpsimd, PE-transpose, evac on ScalarE
        for st in range(ST_):
            x8 = wsc_pool.tile([P, D], FP8, tag="x8", bufs=ST_,
                               name=f"x8_{st}")
            nc.scalar.copy(x8, xn[st])
            tps = psum_tv.tile([P, 2 * D], FP8, tag="tp8", bufs=2,
                               name="tpsx")
            t4 = tps.rearrange("p (k b two) -> p k b two", two=2, b=P)
            for kt in range(KT_):
                nc.tensor.transpose(t4[:, kt, :, 0], x8[:, ts(kt, P)],
                                    ident8)
            nc.scalar.copy(xv[:, :, ds(st * P, P)], t4[:, :, :, 0])

        _load_w8T(nc, wq_all, wsc_pool, psum_tv, wq_nat, ident8,
                  "wq", cast_eng=nc.vector, evac_eng=nc.scalar)
        _load_w8T(nc, wk_all, wsc_pool, psum_tv, wk_nat, ident8,
                  "wk", cast_eng=nc.vector, evac_eng=nc.vector)

        def qk_proj(pr):
            for wv3, bias_t, dest in ((wqv, bq_t, qT), (wkv, bk_t, kT)):
                qps = psum_qk.tile([P, S], F32, tag="qkps", bufs=2,
                                   name="qps")
                for p2 in range(KT_ // 2):
                    for qc in range(0, S, 512):
                        nc.tensor.matmul(
                            qps[:, ds(qc, 512)],
                            lhsT=wv3[:, 2 * p2 : 2 * p2 + 2, ts(pr, P)],
                            rhs=xv[:, 2 * p2 : 2 * p2 + 2, ds(qc, 512)],
                            start=(p2 == 0),
                            stop=(p2 == KT_ // 2 - 1),
                            perf_mode=DR,
                        )
                if use_qkv_bias:
                    nc.vector.tensor_scalar_add(dest[pr], qps,
                                                bias_t[:, pr : pr + 1])
                elif dest is qT:
                    nc.scalar.copy(dest[pr], qps)
                else:
                    nc.vector.tensor_copy(dest[pr], qps)

        qk_proj(0)
        qk_proj(1)
        # wv rode the scalar+gpsimd rings behind wq/wk (issued up top)
        _load_w8T(nc, wv_all, wsc_pool, psum_tv, wv_nat, ident8,
                  "wv", cast_eng=nc.vector, evac_eng=nc.vector)

    # =========== phase 3: attention, two heads at a time ===========
    # QK(2..5) and V are NOT projected up front: they share the scores psum
    # pool and are woven into attention pairs 0-1 (one projection group per
    # score/exp slot), so attention starts ~40us earlier while ScalarE
    # would otherwise idle.
    wdv = wd_all.rearrange("p (k f) -> p k f", f=D)
    with tc.tile_pool(name="expT", bufs=1) as exp_pool, \
         tc.tile_pool(name="den", bufs=1) as den_pool, \
         tc.tile_pool(name="ps_s", bufs=2, space="PSUM") as psum_s, \
         tc.tile_pool(name="ps_ctx", bufs=2, space="PSUM") as psum_ctx:

        def emit_v(st):
            vps = psum_s.tile([P, S], F32, tag="sps", bufs=2, name="vps")
            for p2 in range(KT_ // 2):
                for c0, cw in ((0, 512), (512, 256)):
                    nc.tensor.matmul(
                        vps[:, ds(c0, cw)],
                        lhsT=xv[:, 2 * p2 : 2 * p2 + 2, ts(st, P)],
                        rhs=wvv[:, 2 * p2 : 2 * p2 + 2, ds(c0, cw)],
                        start=(p2 == 0),
                        stop=(p2 == KT_ // 2 - 1),
                        perf_mode=DR,
                    )
            v3 = vps[:, 0:D].rearrange("p (h c) -> p h c", c=DH)
            vdst = vv[:, st // 2, st % 2, :, 0:DH]
            if use_qkv_bias:
                stage = wsc_pool.tile([P, D], F32, tag="vstage", bufs=2,
                                      name="vstage")
                s3 = stage.rearrange("p (h c) -> p h c", c=DH)
                bv3 = bv_bc.rearrange("p (h c) -> p h c", c=DH)
                nc.vector.tensor_add(s3, v3, bv3)
                nc.vector.tensor_copy(vdst, s3)
            else:
                nc.vector.tensor_copy(vdst, v3)

        def emit_qk_half(pr, which):
            wv3, bias_t, dest = ((wqv, bq_t, qT), (wkv, bk_t, kT))[which]
            qps = psum_s.tile([P, S], F32, tag="sps", bufs=2, name="lqps")
            for p2 in range(KT_ // 2):
                for qc in range(0, S, 512):
                    nc.tensor.matmul(
                        qps[:, ds(qc, 512)],
                        lhsT=wv3[:, 2 * p2 : 2 * p2 + 2, ts(pr, P)],
                        rhs=xv[:, 2 * p2 : 2 * p2 + 2, ds(qc, 512)],
                        start=(p2 == 0),
                        stop=(p2 == KT_ // 2 - 1),
                        perf_mode=DR,
                    )
            if use_qkv_bias:
                nc.vector.tensor_scalar_add(dest[pr], qps,
                                            bias_t[:, pr : pr + 1])
            else:
                # DVE for BOTH halves: ScalarE must carry only exps during
                # attention (an evac there stalls the exp pacing stream)
                nc.vector.tensor_copy(dest[pr], qps)

        # Deadline-scheduled work: V(st) is needed by ctx(0, st//2) so V
        # rides every other slot of pair 0; QK(2..5) are needed only by
        # their own pairs, so their halves spread two-per-pair over pairs
        # 1-4. This keeps the per-slot PE load near-uniform instead of
        # cramming all 16 projection groups into pair 0.
        extra_sched = {}
        for k in range(ST_):
            extra_sched[2 * k + 1] = lambda st=k: emit_v(st)
        qkslots = [17, 19, 33, 35, 49, 51, 65, 67]
        qi = 0
        for pr2 in range(2, KT_):
            for which in (0, 1):
                extra_sched[qkslots[qi]] = (
                    lambda pr=pr2, w=which: emit_qk_half(pr, w))
                qi += 1

        def emit_ctx(pend):
            # ctx DoubleRow matmuls for one deferred (pair, jp) group
            pr, jp, cc, ets = pend
            for half in range(2):
                h = 2 * pr + half
                e3 = ets[half].rearrange("p (t q) -> p t q", q=S)
                for qc in range(0, S, 512):
                    nc.tensor.matmul(
                        cc[half][:, ds(qc, 512)],
                        lhsT=vv[:, jp, :, h, :],
                        rhs=e3[:, :, ds(qc, 512)],
                        start=(jp == 0),
                        stop=(jp == 3),
                        perf_mode=DR,
                    )
            if jp == 3:
                emit_den(pr, cc)

        def emit_den(pr, cc):
            # normalize: ctxT = cc[0:64] / cc[64] into ctx_all (fp8)
            for half in range(2):
                h = 2 * pr + half
                kt = h // 2
                den_sb = den_pool.tile([1, S], F32, tag="den_sb", bufs=2)
                nc.vector.tensor_copy(den_sb, cc[half][DH : DH + 1, :])
                rec = den_pool.tile([1, S], F32, tag="rec", bufs=2)
                nc.vector.reciprocal_approx_fast(rec, den_sb)
                recb = den_pool.tile([DH, S], F32, tag="recb", bufs=2)
                nc.gpsimd.partition_broadcast(recb, rec)
                nc.vector.tensor_mul(
                    ctx_all[DH * (h % 2) : DH * (h % 2) + DH, ts(kt, S)],
                    cc[half][0:DH, :], recb)

        pending = None  # deferred ctx group: hides under the next jp's exps
        for pr in range(H // 2):
            if pr == 1:
                # wd arrives via the background permuted path: sync-ring
                # DMA + DVE StreamTranspose (no psum — the attention pools
                # own all 8 banks). DVE has ~10us/pair of slack here.
                _load_wT_dve(nc, wd_all, wsc_pool, wd_ap, nc.sync, "wd")
            cc = [psum_ctx.tile([HB, S], F32, tag="cps", bufs=2,
                                name=f"cps{half}") for half in range(2)]
            et = [None, None]
            for j in range(ST_):
                jp, jh = j // 2, j % 2
                for half in range(2):
                    hp = DH * half
                    if jh == 0:
                        et[half] = exp_pool.tile([P, 2 * S], FP8,
                                                 tag=f"e{half}", bufs=4,
                                                 name=f"e{half}")
                    sps = psum_s.tile([P, S], F32, tag="sps", bufs=2,
                                      name=f"sps{half}")
                    for qc in range(0, S, 512):
                        nc.tensor.matmul(
                            sps[:, ds(qc, 512)],
                            lhsT=kT[pr][hp : hp + DH, ts(j, P)],
                            rhs=qT[pr][hp : hp + DH, ds(qc, 512)],
                            start=True,
                            stop=True,
                        )
                    nc.scalar.activation(
                        et[half][:, ds(jh * S, S)], sps, FT.Exp,
                        bias=(maskT[:, j : j + 1] if use_mask else 0.0),
                        scale=0.125,
                    )
                    job = extra_sched.pop(pr * 16 + j * 2 + half, None)
                    if job:
                        job()
                if jh == 1:
                    if pending is not None:
                        emit_ctx(pending)
                    pending = (pr, jp, cc, (et[0], et[1]))
        emit_ctx(pending)

    # =========== phase 4: dense + residual + layernorm ===========
    # LN stats are batched: each st's STT accumulates its row-sum into one
    # column of a [P, 8] tile, then mu/var/std/rstd are computed once for
    # all 8 seq-tiles (removes ~40 tiny dependency-chained ops from the
    # tail).
    with tc.tile_pool(name="ln", bufs=2) as ln_pool, \
         tc.tile_pool(name="stat", bufs=1) as stat_pool, \
         tc.tile_pool(name="osb", bufs=3) as out_pool, \
         tc.tile_pool(name="ps_o", bufs=2, space="PSUM") as psum_o:

        sums8 = stat_pool.tile([P, ST_], F32, tag="sums8")
        ssq8 = stat_pool.tile([P, ST_], F32, tag="ssq8")
        fulls = []
        for st in range(ST_):
            xr = xn[st]
            if use_dense_bias:
                xb = ln_pool.tile([P, D], F32, tag="xb", bufs=2, name="xb")
                nc.vector.tensor_add(xb, xr, bd_bc)
                xr = xb
            ops = psum_o.tile([P, D], F32, tag="ops", bufs=2)
            for p2 in range(KT_ // 2):
                for c0, cw in ((0, 512), (512, 256)):
                    nc.tensor.matmul(
                        ops[:, ds(c0, cw)],
                        lhsT=cxv[:, 2 * p2 : 2 * p2 + 2, ts(st, P)],
                        rhs=wdv[:, 2 * p2 : 2 * p2 + 2, ds(c0, cw)],
                        start=(p2 == 0),
                        stop=(p2 == KT_ // 2 - 1),
                        perf_mode=DR,
                    )
            # full = dense_out + x, accumulating the row-sum on the fly
            full = ln_pool.tile([P, D], F32, tag="full", bufs=ST_,
                                name=f"full{st}")
            nc.vector.scalar_tensor_tensor(
                out=full, in0=ops, scalar=1.0, in1=xr,
                op0=ALU.mult, op1=ALU.add,
                accum_out=sums8[:, st : st + 1],
            )
            # sum of squares on ScalarE (idle after the exps) — runs in
            # parallel with the DVE residual-STTs; sq is a dead store
            sq = ln_pool.tile([P, D], F32, tag="sq", bufs=2, name="sq")
            nc.scalar.activation(sq, full, FT.Square,
                                 accum_out=ssq8[:, st : st + 1])
            fulls.append(full)

        mu8 = stat_pool.tile([P, ST_], F32, tag="mu8")
        nc.vector.tensor_scalar_mul(mu8, sums8, 1.0 / D)
        mu28 = stat_pool.tile([P, ST_], F32, tag="mu28")
        nc.vector.tensor_mul(mu28, mu8, mu8)
        var8 = stat_pool.tile([P, ST_], F32, tag="var8")
        nc.vector.scalar_tensor_tensor(
            out=var8, in0=ssq8, scalar=1.0 / D, in1=mu28,
            op0=ALU.mult, op1=ALU.subtract,
        )
        std8 = stat_pool.tile([P, ST_], F32, tag="std8")
        nc.scalar.activation(std8, var8, FT.Sqrt, bias=eps_t)
        rstd8 = stat_pool.tile([P, ST_], F32, tag="rstd8")
        nc.vector.reciprocal(rstd8, std8)
        # -mu*rstd, so ScalarE can normalize via Identity(full*rstd + bias)
        nmr8 = stat_pool.tile([P, ST_], F32, tag="nmr8")
        nc.vector.scalar_tensor_tensor(
            out=nmr8, in0=mu8, scalar=-1.0, in1=rstd8,
            op0=ALU.mult, op1=ALU.mult,
        )

        for st in range(ST_):
            osb = out_pool.tile([P, D], F32, tag="osb", name="osb")
            if st % 2 == 0:
                nc.scalar.activation(
                    osb, fulls[st], FT.Identity,
                    bias=nmr8[:, st : st + 1],
                    scale=rstd8[:, st : st + 1],
                )
            else:
                nc.vector.tensor_scalar(
                    out=osb, in0=fulls[st], scalar1=mu8[:, st : st + 1],
                    scalar2=rstd8[:, st : st + 1],
                    op0=ALU.subtract, op1=ALU.mult,
                )
            if use_ln_affine:
                nc.vector.tensor_mul(osb, osb, g_bc)
                nc.vector.tensor_add(osb, osb, b_bc)
            # stripe output DMA over all three rings (3.1MB on one ring
            # would serialize ~21us of tail)
            [nc.sync, nc.scalar, nc.gpsimd][st % 3].dma_start(
                out=out_ap[ts(st, P), :], in_=osb)


def build(flags):
    nc = bacc.Bacc(
        "TRN2", target_bir_lowering=False, debug=False, num_devices=N_CORES
    )
    aps = {}
    for name, shape in (
        ("hidden_states", [S, D]),
        ("attention_mask", [S]),
        ("Wq", [D, D]), ("bq", [D]),
        ("Wk", [D, D]), ("bk", [D]),
        ("Wv", [D, D]), ("bv", [D]),
        ("Wd", [D, D]), ("bd", [D]),
        ("ln_g", [D]), ("ln_b", [D]),
    ):
        aps[name] = nc.dram_tensor(name, shape, F32, kind="ExternalInput").ap()
    out = nc.dram_tensor("out", [S, D], F32, kind="ExternalOutput").ap()

    with tile.TileContext(nc) as tc:
        bert_attn_kernel(
            tc, out,
            aps["hidden_states"], aps["attention_mask"],
            aps["Wq"], aps["bq"], aps["Wk"], aps["bk"],
            aps["Wv"], aps["bv"], aps["Wd"], aps["bd"],
            aps["ln_g"], aps["ln_b"],
            *flags,
        )
    nc.compile()
    return nc


_CACHE = {}
last_results = None  # BassKernelResults of the most recent run (for test.py)


def kernel(**inputs):
    xs = {k: np.ascontiguousarray(np.asarray(v, dtype=np.float32))
          for k, v in inputs.items()}
    B = xs["hidden_states"].shape[0]
    assert B == N_CORES

    flags = (
        bool(np.any(xs["attention_mask"])),
        bool(np.any(xs["bq"]) or np.any(xs["bk"]) or np.any(xs["bv"])),
        bool(np.any(xs["bd"])),
        bool(np.any(xs["ln_g"] != 1.0) or np.any(xs["ln_b"])),
    )
    if flags not in _CACHE:
        _CACHE[flags] = build(flags)
    nc = _CACHE[flags]

    shared = {k: xs[k] for k in
              ("Wq", "bq", "Wk", "bk", "Wv", "bv", "Wd", "bd", "ln_g", "ln_b")}
    in_maps = [
        dict(
            hidden_states=xs["hidden_states"][i],
            attention_mask=np.ascontiguousarray(
                xs["attention_mask"][i].reshape(S)),
            **shared,
        )
        for i in range(N_CORES)
    ]
    trace = bool(int(os.environ.get("BERT_KERNEL_TRACE", "0")))
    res = run_bass_kernel_spmd(
        nc, in_maps, core_ids=list(range(N_CORES)), trace=trace
    )
    global last_results
    last_results = res
    return np.stack([res.results[i]["out"] for i in range(N_CORES)], axis=0)


if __name__ == "__main__":
    rng = np.random.default_rng(0)
    ins = {
        "hidden_states": rng.standard_normal((8, S, D), dtype=np.float32),
        "attention_mask": np.zeros((8, 1, 1, S), np.float32),
        "Wq": rng.standard_normal((D, D), dtype=np.float32) * 0.02,
        "bq": np.zeros(D, np.float32),
        "Wk": rng.standard_normal((D, D), dtype=np.float32) * 0.02,
        "bk": np.zeros(D, np.float32),
        "Wv": rng.standard_normal((D, D), dtype=np.float32) * 0.02,
        "bv": np.zeros(D, np.float32),
        "Wd": rng.standard_normal((D, D), dtype=np.float32) * 0.02,
        "bd": np.zeros(D, np.float32),
        "ln_g": np.ones(D, np.float32),
        "ln_b": np.zeros(D, np.float32),
    }
    out = kernel(**ins)
    print(out.shape, out.dtype, np.abs(out).max())



# revision 10
# speedup vs baseline: 1.0255x; 1.0255x over previous
"""BERT self-attention layer (B=8, S=1024, H=12, Dh=64) on 8 trn2 NeuronCores.

Sharding: pure data-parallel over batch (1 batch item per core, weights
replicated).

Matmul path runs in fp8e4m3 with DoubleRow perf mode where the contraction
is >=256 (QKV projections 6->3 passes, ctx 8->4, dense 6->3).  Scores stay
single-pass fp8 (K=64).  The residual + LN path stays exact fp32; the
residual dominates the output (dense branch is ~1% of it), so fp8 noise in
the attention path dilutes ~100x and the final error is ~1e-4 relative.

Layouts (T = features on partitions):
  x_all  [128, 6*1024] fp8  xT, col = kt*1024 + q        (DR pairs via view)
  w*_all [128, 6*768]  fp8  W^T, col = kt*768 + fo
  qT/kT  6 x [128, 1024] fp8 per head-pair
  vpair  [128, 4*2*1536] fp8  V natural in 128-wide head blocks:
         col = jp*3072 + (j%2)*1536 + 128*h + c; c in 0:64 = values,
         c=64 = ones (denominator row), 65:127 = filler ones feeding unread
         psum rows (DoubleRow ldweights requires stationary width 32/64/128)
  e      [128, 2*1024] fp8 per (half, jp): exp(scores) pairs for DR ctx
  ctx_all[128, 6*1024] fp8  normalized ctx^T, col = kt*1024 + q

Per-core dataflow:
  loads: everything row-major (big DMA packets; the 32x32-permuted loads
         were DMA-packet-bound at ~40us/weight), spread over the three
         DMA-capable queue rings (sync / scalar / gpsimd)
  xT   = ScalarE fp8 cast + PE fp8 transpose (1 cyc/col, psum elem step 2),
         ScalarE evac
  w^T  = DVE fp8 cast + PE fp8 transpose, DVE evac (all four weights up
         front; attention needs all 8 psum banks)
  QT/KT= DR(w^T, xT) per head-pair, DVE evac
  V    = DR(xT, wv^T) natural layout, DVE strided evac into vpair
  per head pair, per j: sT = K^T-slice @ QT (fp8, psum) with the two
         halves' matmuls interleaved (row-groups 0:64 / 64:128 execute
         concurrently in the PE array); e(half0) = ScalarE exp(sT/8+mask),
         e(half1) = DVE bit-trick exp (u8 = sT*log2e + 56 bitcast fp8e4m3,
         log-linear interp error ~4%/elem cancels through the softmax
         denominator)
  ctx  = DR(vpair, e-pairs) accumulated over 4 jp, deferred one jp so the
         matmuls hide under the next exps -> [128, S]; row 64 = denominator
  ctxT = cc[0:64] * bcast(1/sbuf-copy(cc[64]))  (the custom-DVE reciprocal
         reads garbage from PSUM, so copy to SBUF first) -> fp8
  out  = LN(x + DR(ctxT, wd^T))  fused via STT/accum_out
"""

import os
import numpy as np
from contextlib import ExitStack

import concourse.bass as bass
import concourse.bacc as bacc
import concourse.tile as tile
from concourse import mybir
from concourse._compat import with_exitstack
from concourse.bass import ts, ds
from concourse.bass_utils import run_bass_kernel_spmd
from concourse.masks import make_identity

H = 12
DH = 64
D = 768
S = 1024
P = 128
KT_ = D // P  # 6 feature tiles
ST_ = S // P  # 8 sequence tiles
HB = 128  # per-head V block width: 64 value cols + ones col at 64 + 63
          # garbage pad cols (DoubleRow ldweights requires stationary width
          # of exactly 32/64/128; psum rows 65-127 are never read)
VW = H * HB  # 1536
EPS = 1e-12
F32 = mybir.dt.float32
FP8 = mybir.dt.float8e4  # e4m3
U8 = mybir.dt.uint8
FT = mybir.ActivationFunctionType
ALU = mybir.AluOpType
DR = mybir.MatmulPerfMode.DoubleRow
N_CORES = 8
ONE_FP8 = 0x38  # fp8e4m3 encoding of 1.0
LOG2E = 1.4426950408889634
# bit-trick exp (half 1): u8 = s*log2e + B8EXP_BIAS, reinterpreted as
# fp8e4m3 ~= exp(s/8) * 2^-7 up to log-linear interpolation error (~4%,
# cancels through the softmax denominator; any constant factor cancels too)
B8EXP_BIAS = 56.0


def _copy(eng, out, in_):
    # ScalarE spells its copy differently from the vector engines
    if hasattr(eng, "tensor_copy"):
        eng.tensor_copy(out, in_)
    else:
        eng.copy(out, in_)


def _w_dma(nc, scratch, w_ap, dma_eng, tag):
    """Issue the row-major weight DMAs. Call EARLY: issues occupy the
    issuing engine's instruction stream, so they must precede its compute
    work or the transfers start tens of us late."""
    nat = [scratch.tile([P, D], F32, tag="wnat", bufs=3 * KT_,
                        name=f"{tag}n{r}") for r in range(KT_)]
    engs = dma_eng if isinstance(dma_eng, list) else [dma_eng]
    for r in range(KT_):
        engs[r % len(engs)].dma_start(out=nat[r], in_=w_ap[ts(r, P), :])
    return nat


def _load_w8T(nc, dest_all, scratch, psum_pool, nat, ident8, tag,
              cast_eng, evac_eng):
    """Load a [768, 768] f32 DRAM weight row-major (fast, few DMA packets),
    cast to fp8 (cast_eng: any vector engine, SBUF only), PE-transpose in
    fp8 (1 cyc/col; psum output needs element step 2), evac (evac_eng: must
    be DVE or ScalarE — gpsimd can't read PSUM) into dest_all
    ([128, 6*768] fp8, col=kt*768+fo)."""
    n8 = [scratch.tile([P, D], FP8, tag="wn8", bufs=2 * KT_,
                       name=f"{tag}e{r}") for r in range(KT_)]
    for r in range(KT_):
        _copy(cast_eng, n8[r], nat[r])
    for c in range(KT_):
        tp = psum_pool.tile([P, 2 * D], FP8, tag="tp8", bufs=2, name="tp8")
        t4 = tp.rearrange("p (k b two) -> p k b two", two=2, b=P)
        for r in range(KT_):
            nc.tensor.transpose(t4[:, r, :, 0], n8[r][:, ts(c, P)], ident8)
        _copy(evac_eng,
              dest_all[:, ds(c * D, D)].rearrange("p (k b) -> p k b", b=P),
              t4[:, :, :, 0])


def _permuted_src(ap, col0, n_free_blocks):
    """DRAM AP enumerating src[32J+r, col0+c] for r,c in 32x32 blocks, in
    (r, J, c) order — the 32x32-block-permuted load feeding StreamTranspose."""
    rs = ap.ap[0][0]
    return bass.AP(
        tensor=ap.tensor,
        offset=ap.offset + col0,
        ap=[[rs, 32], [32 * rs, n_free_blocks], [1, 32]],
    )


def _load_wT_dve(nc, dest_all, scratch, src_ap, dma_eng, tag):
    """Background weight transpose with NO psum: permuted DMA (packet-rate
    bound, ~40us on the wire — fine for a weight needed late) -> DVE
    StreamTranspose -> DVE fp8 cast into dest_all."""
    for kt in range(KT_):
        perm = scratch.tile([P, D], F32, tag="tsp", bufs=2, name=f"{tag}p")
        p4 = perm.rearrange("(i r) (j c) -> i r j c", r=32, c=32)
        for i in range(4):
            dma_eng.dma_start(
                out=p4[i],
                in_=_permuted_src(src_ap, 128 * kt + 32 * i, D // 32),
            )
        tf = scratch.tile([P, D], F32, tag="tst", bufs=2, name=f"{tag}t")
        nc.vector.transpose(tf, perm)
        nc.vector.tensor_copy(dest_all[:, ds(kt * D, D)], tf)


def _bcast_load(nc, out_tile, vec_ap, n_part):
    """DMA a [N] DRAM vector replicated across n_part partitions."""
    src = bass.AP(
        tensor=vec_ap.tensor,
        offset=vec_ap.offset,
        ap=[[0, n_part]] + [list(d) for d in vec_ap.ap],
    )
    nc.gpsimd.dma_start(out=out_tile, in_=src)


@with_exitstack
def bert_attn_kernel(
    ctx: ExitStack,
    tc: tile.TileContext,
    out_ap: bass.AP,
    x_ap: bass.AP,
    mask_ap: bass.AP,
    wq_ap: bass.AP,
    bq_ap: bass.AP,
    wk_ap: bass.AP,
    bk_ap: bass.AP,
    wv_ap: bass.AP,
    bv_ap: bass.AP,
    wd_ap: bass.AP,
    bd_ap: bass.AP,
    g_ap: bass.AP,
    b_ap: bass.AP,
    use_mask: bool,
    use_qkv_bias: bool,
    use_dense_bias: bool,
    use_ln_affine: bool,
):
    nc = tc.nc

    # ---- persistent pools ----
    const_pool = ctx.enter_context(tc.tile_pool(name="const", bufs=1))
    big_pool = ctx.enter_context(tc.tile_pool(name="big", bufs=1))

    eps_t = const_pool.tile([P, 1], F32)
    nc.vector.memset(eps_t, EPS)
    ident = const_pool.tile([P, P], F32)
    make_identity(nc, ident)
    ident8 = const_pool.tile([P, P], FP8)
    nc.vector.tensor_copy(ident8, ident)

    maskT = mask2T = None
    if use_mask:
        maskT = const_pool.tile([P, ST_], F32)
        nc.sync.dma_start(out=maskT, in_=mask_ap.rearrange("(t p) -> p t", p=P))
        # bias for the DVE bit-trick exp: mask*8*log2e + B8EXP_BIAS
        mask2T = const_pool.tile([P, ST_], F32)
        nc.vector.tensor_scalar(
            out=mask2T, in0=maskT, scalar1=8.0 * LOG2E, scalar2=B8EXP_BIAS,
            op0=ALU.mult, op1=ALU.add)

    bq_t = bk_t = bv_bc = None
    if use_qkv_bias:
        bq_t = const_pool.tile([P, KT_], F32)
        nc.sync.dma_start(out=bq_t, in_=bq_ap.rearrange("(t p) -> p t", p=P))
        bk_t = const_pool.tile([P, KT_], F32)
        nc.sync.dma_start(out=bk_t, in_=bk_ap.rearrange("(t p) -> p t", p=P))
        bv_bc = const_pool.tile([P, D], F32)
        _bcast_load(nc, bv_bc, bv_ap, P)
    bd_bc = None
    if use_dense_bias:
        bd_bc = const_pool.tile([P, D], F32)
        _bcast_load(nc, bd_bc, bd_ap, P)
    g_bc = b_bc = None
    if use_ln_affine:
        g_bc = const_pool.tile([P, D], F32)
        _bcast_load(nc, g_bc, g_ap, P)
        b_bc = const_pool.tile([P, D], F32)
        _bcast_load(nc, b_bc, b_ap, P)

    # persistent data tiles
    x_all = big_pool.tile([P, KT_ * S], FP8, name="x_all")
    xv = x_all.rearrange("p (k q) -> p k q", q=S)
    wq_all = big_pool.tile([P, KT_ * D], FP8, name="wq_all")
    wk_all = big_pool.tile([P, KT_ * D], FP8, name="wk_all")
    wv_all = big_pool.tile([P, KT_ * D], FP8, name="wv_all")
    qT = [big_pool.tile([P, S], FP8, name=f"qT{i}") for i in range(KT_)]
    kT = [big_pool.tile([P, S], FP8, name=f"kT{i}") for i in range(KT_)]
    vpair = big_pool.tile([P, 4 * 2 * VW], FP8, name="vpair")
    vv = vpair.rearrange("p (j t h c) -> p j t h c", t=2, h=H, c=HB)
    xn = [big_pool.tile([P, D], F32, tag="xn", bufs=ST_, name=f"xn{i}")
          for i in range(ST_)]
    ctx_all = big_pool.tile([P, KT_ * S], FP8, name="ctx_all")
    cxv = ctx_all.rearrange("p (k q) -> p k q", q=S)
    wd_all = big_pool.tile([P, KT_ * D], FP8, name="wd_all")

    # Head-block column map: 0:64 = values, 64 = ones (denominator row),
    # 65:127 = filler ones feeding unread psum rows (deterministic, no
    # uninitialized weights entering the PE).
    for jp in range(4):
        nc.gpsimd.memset(vv[:, jp, :, :, DH:HB].bitcast(U8), ONE_FP8)

    # =========== phase 1: x + weight loads (row-major, 3 queue rings) ======
    # x striped across all three rings (it gates everything), then wq/wk
    # striped two rings each (they gate attention start), wv behind them;
    # wd comes via the background permuted path on sync during attention.
    # Casts/evacs are spread over gpsimd (SBUF-only), ScalarE and DVE so no
    # single engine serializes the lead-in.
    rings = [nc.sync, nc.scalar, nc.gpsimd]
    for st in range(ST_):
        rings[st % 3].dma_start(out=xn[st], in_=x_ap[ts(st, P), :])

    wsc_pool = ctx.enter_context(tc.tile_pool(name="wsc", bufs=1))
    wq_nat = _w_dma(nc, wsc_pool, wq_ap, [nc.scalar, nc.sync], "wq")
    wk_nat = _w_dma(nc, wsc_pool, wk_ap, [nc.gpsimd, nc.sync], "wk")
    wv_nat = _w_dma(nc, wsc_pool, wv_ap, [nc.scalar, nc.gpsimd], "wv")

    # =========== phase 2: transposes + QKV projections (DoubleRow) ========
    wqv = wq_all.rearrange("p (k f) -> p k f", f=D)
    wkv = wk_all.rearrange("p (k f) -> p k f", f=D)
    wvv = wv_all.rearrange("p (k f) -> p k f", f=D)

    with tc.tile_pool(name="ps_tv", bufs=1, space="PSUM") as psum_tv, \
         tc.tile_pool(name="ps_qk", bufs=2, space="PSUM") as psum_qk:

        # xT first in the PE stream (x lands before the weights): cast to
        # fp8 on gpsimd, PE-transpose, evac on ScalarE
        for st in range(ST_):
            x8 = wsc_pool.tile([P, D], FP8, tag="x8", bufs=ST_,
                               name=f"x8_{st}")
            nc.scalar.copy(x8, xn[st])
            tps = psum_tv.tile([P, 2 * D], FP8, tag="tp8", bufs=2,
                               name="tpsx")
            t4 = tps.rearrange("p (k b two) -> p k b two", two=2, b=P)
            for kt in range(KT_):
                nc.tensor.transpose(t4[:, kt, :, 0], x8[:, ts(kt, P)],
                                    ident8)
            nc.scalar.copy(xv[:, :, ds(st * P, P)], t4[:, :, :, 0])

        _load_w8T(nc, wq_all, wsc_pool, psum_tv, wq_nat, ident8,
                  "wq", cast_eng=nc.vector, evac_eng=nc.scalar)
        _load_w8T(nc, wk_all, wsc_pool, psum_tv, wk_nat, ident8,
                  "wk", cast_eng=nc.vector, evac_eng=nc.vector)

        def qk_proj(pr):
            for wv3, bias_t, dest in ((wqv, bq_t, qT), (wkv, bk_t, kT)):
                qps = psum_qk.tile([P, S], F32, tag="qkps", bufs=2,
                                   name="qps")
                for p2 in range(KT_ // 2):
                    for qc in range(0, S, 512):
                        nc.tensor.matmul(
                            qps[:, ds(qc, 512)],
                            lhsT=wv3[:, 2 * p2 : 2 * p2 + 2, ts(pr, P)],
                            rhs=xv[:, 2 * p2 : 2 * p2 + 2, ds(qc, 512)],
                            start=(p2 == 0),
                            stop=(p2 == KT_ // 2 - 1),
                            perf_mode=DR,
                        )
                if use_qkv_bias:
                    nc.vector.tensor_scalar_add(dest[pr], qps,
                                                bias_t[:, pr : pr + 1])
                elif dest is qT:
                    nc.scalar.copy(dest[pr], qps)
                else:
                    nc.vector.tensor_copy(dest[pr], qps)

        qk_proj(0)
        qk_proj(1)
        # wv rode the scalar+gpsimd rings behind wq/wk (issued up top)
        _load_w8T(nc, wv_all, wsc_pool, psum_tv, wv_nat, ident8,
                  "wv", cast_eng=nc.vector, evac_eng=nc.vector)

    # =========== phase 3: attention, two heads at a time ===========
    # QK(2..5) and V are NOT projected up front: they share the scores psum
    # pool and are woven into attention pairs 0-1 (one projection group per
    # score/exp slot), so attention starts ~40us earlier while ScalarE
    # would otherwise idle.
    wdv = wd_all.rearrange("p (k f) -> p k f", f=D)
    with tc.tile_pool(name="expT", bufs=1) as exp_pool, \
         tc.tile_pool(name="den", bufs=1) as den_pool, \
         tc.tile_pool(name="ps_s", bufs=2, space="PSUM") as psum_s, \
         tc.tile_pool(name="ps_ctx", bufs=2, space="PSUM") as psum_ctx:

        def emit_v(st):
            vps = psum_s.tile([P, S], F32, tag="sps", bufs=2, name="vps")
            for p2 in range(KT_ // 2):
                for c0, cw in ((0, 512), (512, 256)):
                    nc.tensor.matmul(
                        vps[:, ds(c0, cw)],
                        lhsT=xv[:, 2 * p2 : 2 * p2 + 2, ts(st, P)],
                        rhs=wvv[:, 2 * p2 : 2 * p2 + 2, ds(c0, cw)],
                        start=(p2 == 0),
                        stop=(p2 == KT_ // 2 - 1),
                        perf_mode=DR,
                    )
            v3 = vps[:, 0:D].rearrange("p (h c) -> p h c", c=DH)
            vdst = vv[:, st // 2, st % 2, :, 0:DH]
            if use_qkv_bias:
                stage = wsc_pool.tile([P, D], F32, tag="vstage", bufs=2,
                                      name="vstage")
                s3 = stage.rearrange("p (h c) -> p h c", c=DH)
                bv3 = bv_bc.rearrange("p (h c) -> p h c", c=DH)
                nc.vector.tensor_add(s3, v3, bv3)
                nc.vector.tensor_copy(vdst, s3)
            else:
                # alternate evac engine: all 8 V groups land in pair 0's
                # slots, so neither ScalarE (exps) nor DVE (bit-exps)
                # should take them all
                if st % 2 == 0:
                    nc.scalar.copy(vdst, v3)
                else:
                    nc.vector.tensor_copy(vdst, v3)

        def emit_qk_half(pr, which):
            wv3, bias_t, dest = ((wqv, bq_t, qT), (wkv, bk_t, kT))[which]
            qps = psum_s.tile([P, S], F32, tag="sps", bufs=2, name="lqps")
            for p2 in range(KT_ // 2):
                for qc in range(0, S, 512):
                    nc.tensor.matmul(
                        qps[:, ds(qc, 512)],
                        lhsT=wv3[:, 2 * p2 : 2 * p2 + 2, ts(pr, P)],
                        rhs=xv[:, 2 * p2 : 2 * p2 + 2, ds(qc, 512)],
                        start=(p2 == 0),
                        stop=(p2 == KT_ // 2 - 1),
                        perf_mode=DR,
                    )
            if use_qkv_bias:
                nc.vector.tensor_scalar_add(dest[pr], qps,
                                            bias_t[:, pr : pr + 1])
            else:
                # ScalarE: DVE carries the half-1 bit-trick exps now
                nc.scalar.copy(dest[pr], qps)

        # Deadline-scheduled work: V(st) is needed by ctx(0, st//2) so V
        # rides every other slot of pair 0; QK(2..5) are needed only by
        # their own pairs, so their halves spread two-per-pair over pairs
        # 1-4. This keeps the per-slot PE load near-uniform instead of
        # cramming all 16 projection groups into pair 0.
        extra_sched = {}
        for k in range(ST_):
            extra_sched[2 * k + 1] = lambda st=k: emit_v(st)
        qkslots = [17, 19, 33, 35, 49, 51, 65, 67]
        qi = 0
        for pr2 in range(2, KT_):
            for which in (0, 1):
                extra_sched[qkslots[qi]] = (
                    lambda pr=pr2, w=which: emit_qk_half(pr, w))
                qi += 1

        def emit_ctx(pend):
            # ctx DoubleRow matmuls for one deferred (pair, jp) group
            pr, jp, cc, ets = pend
            for half in range(2):
                h = 2 * pr + half
                e3 = ets[half].rearrange("p (t q) -> p t q", q=S)
                for qc in range(0, S, 512):
                    nc.tensor.matmul(
                        cc[half][:, ds(qc, 512)],
                        lhsT=vv[:, jp, :, h, :],
                        rhs=e3[:, :, ds(qc, 512)],
                        start=(jp == 0),
                        stop=(jp == 3),
                        perf_mode=DR,
                    )
            if jp == 3:
                emit_den(pr, cc)

        def emit_den(pr, cc):
            # normalize: ctxT = cc[0:64] / cc[64] into ctx_all (fp8)
            for half in range(2):
                h = 2 * pr + half
                kt = h // 2
                den_sb = den_pool.tile([1, S], F32, tag="den_sb", bufs=2)
                nc.vector.tensor_copy(den_sb, cc[half][DH : DH + 1, :])
                rec = den_pool.tile([1, S], F32, tag="rec", bufs=2)
                nc.vector.reciprocal_approx_fast(rec, den_sb)
                recb = den_pool.tile([DH, S], F32, tag="recb", bufs=2)
                nc.gpsimd.partition_broadcast(recb, rec)
                nc.vector.tensor_mul(
                    ctx_all[DH * (h % 2) : DH * (h % 2) + DH, ts(kt, S)],
                    cc[half][0:DH, :], recb)

        pending = None  # deferred ctx group: hides under the next jp's exps
        for pr in range(H // 2):
            if pr == 1:
                # wd arrives via the background permuted path: sync-ring
                # DMA + DVE StreamTranspose (no psum — the attention pools
                # own all 8 banks). DVE has ~10us/pair of slack here.
                _load_wT_dve(nc, wd_all, wsc_pool, wd_ap, nc.sync, "wd")
            cc = [psum_ctx.tile([HB, S], F32, tag="cps", bufs=2,
                                name=f"cps{half}") for half in range(2)]
            et = [None, None]
            for j in range(ST_):
                jp, jh = j // 2, j % 2
                if jh == 0:
                    for half in range(2):
                        et[half] = exp_pool.tile([P, 2 * S], FP8,
                                                 tag=f"e{half}", bufs=4,
                                                 name=f"e{half}")
                # Interleave the two halves' score matmuls: lhsT base
                # partitions 0 / 64 map to distinct PE row-groups, so
                # adjacent matmuls execute concurrently in the array.
                sps = [psum_s.tile([P, S], F32, tag="sps", bufs=2,
                                   name=f"sps{half}") for half in range(2)]
                for qc in range(0, S, 512):
                    for half in range(2):
                        hp = DH * half
                        nc.tensor.matmul(
                            sps[half][:, ds(qc, 512)],
                            lhsT=kT[pr][hp : hp + DH, ts(j, P)],
                            rhs=qT[pr][hp : hp + DH, ds(qc, 512)],
                            start=True,
                            stop=True,
                        )
                # Split the exps across engines: half 0 = true exp on
                # ScalarE; half 1 = bit-trick exp on DVE (u8 = s*log2e +
                # bias, reinterpreted as fp8e4m3; the log-linear interp
                # error is ~4% per element and cancels through softmax
                # normalization — same order as the fp8 e quantization).
                nc.scalar.activation(
                    et[0][:, ds(jh * S, S)], sps[0], FT.Exp,
                    bias=(maskT[:, j : j + 1] if use_mask else 0.0),
                    scale=0.125,
                )
                nc.vector.tensor_scalar(
                    out=et[1][:, ds(jh * S, S)].bitcast(U8),
                    in0=sps[1],
                    scalar1=LOG2E,
                    scalar2=(mask2T[:, j : j + 1] if use_mask
                             else B8EXP_BIAS),
                    op0=ALU.mult,
                    op1=ALU.add,
                )
                for half in range(2):
                    job = extra_sched.pop(pr * 16 + j * 2 + half, None)
                    if job:
                        job()
                if jh == 1:
                    if pending is not None:
                        emit_ctx(pending)
                    pending = (pr, jp, cc, (et[0], et[1]))
        emit_ctx(pending)

    # =========== phase 4: dense + residual + layernorm ===========
    # LN stats are batched per 4 seq-tiles: each st's STT accumulates its
    # row-sum into one column of a [P, 8] tile; stats + normalize + output
    # DMA for st 0-3 overlap the dense matmuls of st 4-7.
    with tc.tile_pool(name="ln", bufs=2) as ln_pool, \
         tc.tile_pool(name="stat", bufs=1) as stat_pool, \
         tc.tile_pool(name="osb", bufs=3) as out_pool, \
         tc.tile_pool(name="ps_o", bufs=2, space="PSUM") as psum_o:

        sums8 = stat_pool.tile([P, ST_], F32, tag="sums8")
        ssq8 = stat_pool.tile([P, ST_], F32, tag="ssq8")
        mu8 = stat_pool.tile([P, ST_], F32, tag="mu8")
        mu28 = stat_pool.tile([P, ST_], F32, tag="mu28")
        var8 = stat_pool.tile([P, ST_], F32, tag="var8")
        std8 = stat_pool.tile([P, ST_], F32, tag="std8")
        rstd8 = stat_pool.tile([P, ST_], F32, tag="rstd8")
        nmr8 = stat_pool.tile([P, ST_], F32, tag="nmr8")
        fulls = []

        def emit_dense(st):
            xr = xn[st]
            if use_dense_bias:
                xb = ln_pool.tile([P, D], F32, tag="xb", bufs=2, name="xb")
                nc.vector.tensor_add(xb, xr, bd_bc)
                xr = xb
            ops = psum_o.tile([P, D], F32, tag="ops", bufs=2)
            for p2 in range(KT_ // 2):
                for c0, cw in ((0, 512), (512, 256)):
                    nc.tensor.matmul(
                        ops[:, ds(c0, cw)],
                        lhsT=cxv[:, 2 * p2 : 2 * p2 + 2, ts(st, P)],
                        rhs=wdv[:, 2 * p2 : 2 * p2 + 2, ds(c0, cw)],
                        start=(p2 == 0),
                        stop=(p2 == KT_ // 2 - 1),
                        perf_mode=DR,
                    )
            # full = dense_out + x, accumulating the row-sum on the fly
            full = ln_pool.tile([P, D], F32, tag="full", bufs=ST_,
                                name=f"full{st}")
            nc.vector.scalar_tensor_tensor(
                out=full, in0=ops, scalar=1.0, in1=xr,
                op0=ALU.mult, op1=ALU.add,
                accum_out=sums8[:, st : st + 1],
            )
            # sum of squares on ScalarE (idle after the exps) — runs in
            # parallel with the DVE residual-STTs; sq is a dead store
            sq = ln_pool.tile([P, D], F32, tag="sq", bufs=2, name="sq")
            nc.scalar.activation(sq, full, FT.Square,
                                 accum_out=ssq8[:, st : st + 1])
            fulls.append(full)

        def emit_stats(c0, cn):
            sl = ds(c0, cn)
            nc.vector.tensor_scalar_mul(mu8[:, sl], sums8[:, sl], 1.0 / D)
            nc.vector.tensor_mul(mu28[:, sl], mu8[:, sl], mu8[:, sl])
            nc.vector.scalar_tensor_tensor(
                out=var8[:, sl], in0=ssq8[:, sl], scalar=1.0 / D,
                in1=mu28[:, sl], op0=ALU.mult, op1=ALU.subtract,
            )
            nc.scalar.activation(std8[:, sl], var8[:, sl], FT.Sqrt,
                                 bias=eps_t)
            nc.vector.reciprocal(rstd8[:, sl], std8[:, sl])
            # -mu*rstd: ScalarE normalizes via Identity(full*rstd + bias)
            nc.vector.scalar_tensor_tensor(
                out=nmr8[:, sl], in0=mu8[:, sl], scalar=-1.0,
                in1=rstd8[:, sl], op0=ALU.mult, op1=ALU.mult,
            )

        def emit_norm(st):
            osb = out_pool.tile([P, D], F32, tag="osb", name="osb")
            if st % 2 == 0:
                nc.scalar.activation(
                    osb, fulls[st], FT.Identity,
                    bias=nmr8[:, st : st + 1],
                    scale=rstd8[:, st : st + 1],
                )
            else:
                nc.vector.tensor_scalar(
                    out=osb, in0=fulls[st], scalar1=mu8[:, st : st + 1],
                    scalar2=rstd8[:, st : st + 1],
                    op0=ALU.subtract, op1=ALU.mult,
                )
            if use_ln_affine:
                nc.vector.tensor_mul(osb, osb, g_bc)
                nc.vector.tensor_add(osb, osb, b_bc)
            # stripe output DMA over all three rings (3.1MB on one ring
            # would serialize ~21us of tail)
            [nc.sync, nc.scalar, nc.gpsimd][st % 3].dma_start(
                out=out_ap[ts(st, P), :], in_=osb)

        for st in range(4):
            emit_dense(st)
        emit_stats(0, 4)
        for st in range(4, ST_):
            emit_dense(st)
        for st in range(4):
            emit_norm(st)
        emit_stats(4, 4)
        for st in range(4, ST_):
            emit_norm(st)


def build(flags):
    nc = bacc.Bacc(
        "TRN2", target_bir_lowering=False, debug=False, num_devices=N_CORES
    )
    aps = {}
    for name, shape in (
        ("hidden_states", [S, D]),
        ("attention_mask", [S]),
        ("Wq", [D, D]), ("bq", [D]),
        ("Wk", [D, D]), ("bk", [D]),
        ("Wv", [D, D]), ("bv", [D]),
        ("Wd", [D, D]), ("bd", [D]),
        ("ln_g", [D]), ("ln_b", [D]),
    ):
        aps[name] = nc.dram_tensor(name, shape, F32, kind="ExternalInput").ap()
    out = nc.dram_tensor("out", [S, D], F32, kind="ExternalOutput").ap()

    with tile.TileContext(nc) as tc:
        bert_attn_kernel(
            tc, out,
            aps["hidden_states"], aps["attention_mask"],
            aps["Wq"], aps["bq"], aps["Wk"], aps["bk"],
            aps["Wv"], aps["bv"], aps["Wd"], aps["bd"],
            aps["ln_g"], aps["ln_b"],
            *flags,
        )
    nc.compile()
    return nc


_CACHE = {}
last_results = None  # BassKernelResults of the most recent run (for test.py)


def kernel(**inputs):
    xs = {k: np.ascontiguousarray(np.asarray(v, dtype=np.float32))
          for k, v in inputs.items()}
    B = xs["hidden_states"].shape[0]
    assert B == N_CORES

    flags = (
        bool(np.any(xs["attention_mask"])),
        bool(np.any(xs["bq"]) or np.any(xs["bk"]) or np.any(xs["bv"])),
        bool(np.any(xs["bd"])),
        bool(np.any(xs["ln_g"] != 1.0) or np.any(xs["ln_b"])),
    )
    if flags not in _CACHE:
        _CACHE[flags] = build(flags)
    nc = _CACHE[flags]

    shared = {k: xs[k] for k in
              ("Wq", "bq", "Wk", "bk", "Wv", "bv", "Wd", "bd", "ln_g", "ln_b")}
    in_maps = [
        dict(
            hidden_states=xs["hidden_states"][i],
            attention_mask=np.ascontiguousarray(
                xs["attention_mask"][i].reshape(S)),
            **shared,
        )
        for i in range(N_CORES)
    ]
    trace = bool(int(os.environ.get("BERT_KERNEL_TRACE", "0")))
    res = run_bass_kernel_spmd(
        nc, in_maps, core_ids=list(range(N_CORES)), trace=trace
    )
    global last_results
    last_results = res
    return np.stack([res.results[i]["out"] for i in range(N_CORES)], axis=0)


if __name__ == "__main__":
    rng = np.random.default_rng(0)
    ins = {
        "hidden_states": rng.standard_normal((8, S, D), dtype=np.float32),
        "attention_mask": np.zeros((8, 1, 1, S), np.float32),
        "Wq": rng.standard_normal((D, D), dtype=np.float32) * 0.02,
        "bq": np.zeros(D, np.float32),
        "Wk": rng.standard_normal((D, D), dtype=np.float32) * 0.02,
        "bk": np.zeros(D, np.float32),
        "Wv": rng.standard_normal((D, D), dtype=np.float32) * 0.02,
        "bv": np.zeros(D, np.float32),
        "Wd": rng.standard_normal((D, D), dtype=np.float32) * 0.02,
        "bd": np.zeros(D, np.float32),
        "ln_g": np.ones(D, np.float32),
        "ln_b": np.zeros(D, np.float32),
    }
    out = kernel(**ins)
    print(out.shape, out.dtype, np.abs(out).max())

